# revision 41
# baseline (speedup 1.0000x reference)
"""Trainium2 Bass kernel for Memorynet (KNN-interp + 1x1-conv MLP).

Strategy: pure data parallel over batch (32 batches -> 8 cores x 4).
Per batch, per 128-token tile:
  S = 2*p1@p2.T - |p2|^2  (one K=21 bf16-split matmul into PSUM, [128 tok, 512 n2])
  top-8 S values via DVE max8 (top-3 used; m4 used for the mask threshold)
  Z = sum_k 1/(p1sq+eps-S_k); the normalized-weight row is built WITHOUT
  index extraction via the D' trick:
      D'_j  = Z*(p1sq+eps) - Z*S_j          (one ScalarE activation pass)
      tau   = Z*(p1sq+eps) - Z*(m3+m4)/2    (midpoint threshold, robust)
      A_j   = (D'_j <= tau) / D'_j          (one GpSimd fused pass -> bf16)
  so A_j = 1[S_j in top-3] * (1/dist_j)/Z exactly.
  A.T via DMA transpose (per 4-tile half); recvT-h1 accumulated in PSUM via
  G1c.T @ ATc matmuls (G1 = f2 @ W1r.T precomputed on host).
MLP is feature-major bf16; BN+ReLU folded into ScalarE activation.
Software-pipelined with depth-2 skew and fine-grain interleave.
"""

import sys

sys.path.insert(0, "/opt/trn_rl_repo")

import numpy as np
import ml_dtypes

import concourse.bass as bass
import concourse.bacc as bacc_mod
import concourse.mybir as mybir
from concourse.tile import TileContext
from concourse.bass_utils import run_bass_kernel_spmd

EPS_DIST = 1e-8  # matches reference; the Z-normalized reciprocal is
                 # self-protecting against fp32 rounding of tiny distances
EPS_BN = 1e-5
NCORES = 8
BPC = 4  # batches per core
N1, N2, C1, C2 = 2048, 512, 128, 256
CIN, H1, H2 = C1 + C2, 256, 128
NT = N1 // 128   # 16 token tiles / batch
PAIR = 8         # token tiles per KNN front (1024 tokens)
NPP = NT // PAIR # 2 pairs per batch

f32 = mybir.dt.float32
bf16 = mybir.dt.bfloat16
u32 = mybir.dt.uint32

AT = mybir.ActivationFunctionType
OP = mybir.AluOpType
AX = mybir.AxisListType


KS = 24  # split-matmul contraction rows: 18 products + 3*(-|p2|^2) + 3*(-(|p1|^2+eps))


def build_bass():
    nc = bacc_mod.Bacc()
    p1e = nc.declare_dram_parameter("p1e", [BPC, KS, N1], bf16, isOutput=False)
    rhs4 = nc.declare_dram_parameter("rhs4", [BPC, KS, N2], bf16, isOutput=False)
    f1T = nc.declare_dram_parameter("f1T", [BPC, C1, N1], bf16, isOutput=False)
    g1s = [
        nc.declare_dram_parameter(f"g1_{b}", [N2, H1], bf16, isOutput=False)
        for b in range(BPC)
    ]
    W1fd = nc.declare_dram_parameter("W1fT", [C1, H1], bf16, isOutput=False)
    W2Td = nc.declare_dram_parameter("W2T", [H1, H2], bf16, isOutput=False)
    sb1d = nc.declare_dram_parameter("sb1", [H1, 2], f32, isOutput=False)
    sb2d = nc.declare_dram_parameter("sb2", [H2, 2], f32, isOutput=False)
    outT = nc.declare_dram_parameter("outT", [BPC, H2, N1], f32, isOutput=True)

    with TileContext(nc) as tc:
        with (
            tc.tile_pool(name="const", bufs=1) as cpool,
            tc.tile_pool(name="batch", bufs=4) as bpool,
            tc.tile_pool(name="math", bufs=3) as gpool,
            tc.tile_pool(name="idxp", bufs=3) as idxpool,
            tc.tile_pool(name="dpp", bufs=4) as dppool,
            tc.tile_pool(name="at", bufs=4) as gkpool,
            tc.tile_pool(name="ab", bufs=4) as dpool,
            tc.tile_pool(name="xg", bufs=4) as xpool,
            tc.tile_pool(name="ps_s", bufs=5, space="PSUM") as ps_s,
            tc.tile_pool(name="ps_mlp", bufs=2, space="PSUM") as ps_mlp,
        ):
            # ---- constants ----
            W1f = cpool.tile([C1, H1], bf16)
            nc.sync.dma_start(out=W1f[:], in_=W1fd[:, :])
            W2T = [cpool.tile([128, H2], bf16, tag=f"w2_{k}", name=f"w2_{k}") for k in range(2)]
            for k in range(2):
                nc.sync.dma_start(out=W2T[k][:], in_=W2Td[128 * k:128 * (k + 1), :])
            sb1 = [cpool.tile([128, 2], f32, tag=f"sb1_{k}", name=f"sb1_{k}") for k in range(2)]
            for k in range(2):
                nc.sync.dma_start(out=sb1[k][:], in_=sb1d[128 * k:128 * (k + 1), :])
            sb2 = cpool.tile([128, 2], f32)
            nc.sync.dma_start(out=sb2[:], in_=sb2d[:, :])

            # ---- per-batch persistent inputs ----
            bstate = {}
            for b in range(BPC):
                p1eb = bpool.tile([KS, N1], bf16, tag="p1eb")
                nc.sync.dma_start(out=p1eb[:], in_=p1e[b, :, :])
                rhsb = bpool.tile([KS, N2], bf16, tag="rhsb")
                nc.sync.dma_start(out=rhsb[:], in_=rhs4[b, :, :])
                g1sb = bpool.tile([128, 4, H1], bf16, tag="g1sb")
                nc.sync.dma_start(
                    out=g1sb[:], in_=g1s[b][:, :].rearrange("(c p) d -> p c d", p=128)
                )
                f1b = bpool.tile([C1, N1], bf16, tag="f1b")
                nc.scalar.dma_start(out=f1b[:], in_=f1T[b, :, :])
                bstate[b] = (p1eb, rhsb, g1sb, f1b)

            def knn_tile(b, gp, t, maxg, sps):
                """One negated-distance matmul (out = -dist) + top-8."""
                p1eb, rhsb, _, _ = bstate[b]
                tau_ = PAIR * gp + t
                Sp = ps_s.tile([128, N2], f32, tag="Sp")
                nc.tensor.matmul(
                    out=Sp[:],
                    lhsT=p1eb[:, 128 * tau_:128 * (tau_ + 1)],
                    rhs=rhsb[:],
                    start=True,
                    stop=True,
                )
                nc.vector.max(out=maxg[:, t, :], in_=Sp[:])
                sps[t] = Sp

            def knn_half(b, gp, h, maxg, sps, ATt, Ag):
                """Weight math + reciprocal weights + mask for 4 tiles.

                maxg holds q = -dist (top-8, descending = 3 nearest first).
                Zneg = sum_k 1/q_k = -Z.  Scalar computes Rp = 1/(Zneg*q)
                = 1/(Z*d) for every position; DVE masks to the top-3 via
                q >= (q3+q4)/2.
                """
                sl = slice(4 * h, 4 * h + 4)
                recd = gpool.tile([128, 4, 3], f32, tag="recd")
                nc.vector.reciprocal(out=recd[:], in_=maxg[:, sl, 0:3])
                Zneg = gpool.tile([128, 4], f32, tag="Zneg")
                nc.vector.reduce_sum(out=Zneg[:], in_=recd[:], axis=AX.X)
                s34 = gpool.tile([128, 4], f32, tag="s34")
                nc.vector.tensor_tensor(
                    out=s34[:], in0=maxg[:, sl, 2], in1=maxg[:, sl, 3], op=OP.add
                )
                tau = gpool.tile([128, 4], f32, tag="tau")
                nc.vector.tensor_scalar_mul(tau[:], s34[:], 0.5)
                for j in range(4):
                    t = 4 * h + j
                    Rp = dppool.tile([128, N2], f32, tag="Rp")
                    nc.scalar.add_instruction(
                        mybir.InstActivation(
                            name=nc.scalar.bass.get_next_instruction_name(),
                            func=AT.Reciprocal,
                            ins=[
                                nc.scalar.lower_ap(sps[t][:]),
                                mybir.ImmediateValue(dtype=f32, value=0.0),
                                nc.scalar.lower_ap(Zneg[:, j:j + 1]),
                                mybir.ImmediateValue(dtype=f32, value=0.0),
                            ],
                            outs=[nc.scalar.lower_ap(Rp[:])],
                        )
                    )
                    # min(mask, Rp): selected -> Rp (weights are <= 1),
                    # masked -> 0; IEEE minNum suppresses NaN/inf from the
                    # reciprocal of near-zero/negative distances, yielding
                    # the correct w=1 limit for degenerate nearest points.
                    nc.vector.scalar_tensor_tensor(
                        out=Ag[:, t, :],
                        in0=sps[t][:],
                        scalar=tau[:, j:j + 1],
                        in1=Rp[:],
                        op0=OP.is_ge,
                        op1=OP.min,
                    )
                nc.sync.dma_start_transpose(
                    out=ATt[:, 16 * h:16 * (h + 1), :], in_=Ag[:, sl, :]
                )

            def mlp_chunks(b, gp, ATt):
                """6 issue-chunks: (g0,m0) (g0,m1) (g0,L2) (g1,m0) (g1,m1) (g1,L2)."""
                _, _, g1sb, f1b = bstate[b]
                ATv = ATt[:].rearrange("p (t c) r -> p c t r", c=4)
                chunks = []
                for gsub in range(2):
                    g = 2 * gp + gsub
                    h1 = [
                        xpool.tile([128, 512], bf16, tag=f"h1_{m}", name=f"h1_{m}")
                        for m in range(2)
                    ]

                    def l1(m, gsub=gsub, g=g, h1=h1):
                        l1p = ps_mlp.tile([128, 512], f32, tag="l1p")
                        nc.tensor.matmul(
                            out=l1p[:],
                            lhsT=W1f[:, 128 * m:128 * (m + 1)],
                            rhs=f1b[:, 512 * g:512 * (g + 1)],
                            start=True,
                            stop=False,
                        )
                        for c in range(4):
                            nc.tensor.matmul(
                                out=l1p[:],
                                lhsT=g1sb[:, c, 128 * m:128 * (m + 1)],
                                rhs=ATv[:, c, 4 * gsub:4 * gsub + 4, :],
                                start=False,
                                stop=(c == 3),
                            )
                        nc.scalar.activation(
                            out=h1[m][:],
                            in_=l1p[:],
                            func=AT.Relu,
                            scale=sb1[m][:, 0:1],
                            bias=sb1[m][:, 1:2],
                        )

                    def l2(g=g, h1=h1):
                        l2p = ps_mlp.tile([128, 512], f32, tag="l2p", bufs=1)
                        for kk in range(2):
                            nc.tensor.matmul(
                                out=l2p[:],
                                lhsT=W2T[kk][:],
                                rhs=h1[kk][:],
                                start=(kk == 0),
                                stop=(kk == 1),
                            )
                        o = xpool.tile([128, 512], f32, tag="osb")
                        nc.scalar.activation(
                            out=o[:],
                            in_=l2p[:],
                            func=AT.Relu,
                            scale=sb2[:, 0:1],
                            bias=sb2[:, 1:2],
                        )
                        nc.sync.dma_start(
                            out=outT[b, :, 512 * g:512 * (g + 1)], in_=o[:]
                        )

                    chunks.append(lambda l1=l1: l1(0))
                    chunks.append(lambda l1=l1: l1(1))
                    chunks.append(l2)
                return chunks

            # ---- software pipeline with fine-grain interleave ----
            iters = [(b, gp) for gp in range(NPP) for b in range(BPC)]
            DEPTH = 3
            pend = []
            ready = []
            for i, (b, gp) in enumerate(iters):
                maxg = idxpool.tile([128, PAIR, 8], f32, tag="maxg")
                sps = {}
                Ag = dpool.tile([128, PAIR, N2], bf16, tag="A")
                ATt = gkpool.tile([128, PAIR * 4, 128], bf16, tag="ATt")
                chunks = mlp_chunks(*pend[i - DEPTH]) if i >= DEPTH else []
                for t in range(4):
                    knn_tile(b, gp, t, maxg, sps)
                knn_half(b, gp, 0, maxg, sps, ATt, Ag)
                if chunks:
                    chunks[0]()
                    chunks[1]()
                knn_tile(b, gp, 4, maxg, sps)
                if chunks:
                    chunks[2]()
                knn_tile(b, gp, 5, maxg, sps)
                if chunks:
                    chunks[3]()
                knn_tile(b, gp, 6, maxg, sps)
                if chunks:
                    chunks[4]()
                knn_tile(b, gp, 7, maxg, sps)
                knn_half(b, gp, 1, maxg, sps, ATt, Ag)
                if chunks:
                    chunks[5]()
                pend.append((b, gp, ATt))
            flush = [mlp_chunks(*pend[j]) for j in range(len(iters) - DEPTH, len(iters))]
            for k in range(max(len(f) for f in flush)):
                for f in flush:
                    if k < len(f):
                        f[k]()

    nc.compile()
    return nc


_CACHE = {}


def _get_nc():
    if "nc" not in _CACHE:
        _CACHE["nc"] = build_bass()
    return _CACHE["nc"]


def _prep_core(inputs, c):
    """Host-side prep of one core's input map (batches 4c..4c+4)."""
    sl = slice(BPC * c, BPC * (c + 1))
    p1 = inputs["points_1"][sl]     # [4, N1, 3]
    p2 = inputs["points_2"][sl]     # [4, N2, 3]
    f1 = inputs["features_1"][sl]   # [4, N1, C1]
    f2 = inputs["features_2"][sl]   # [4, N2, C2]

    def split3(x):
        a = x.astype(ml_dtypes.bfloat16)
        r = x - a.astype(np.float32)
        bb = r.astype(ml_dtypes.bfloat16)
        cc = (r - bb.astype(np.float32)).astype(ml_dtypes.bfloat16)
        return a, bb, cc

    p1T = np.transpose(p1, (0, 2, 1)).astype(np.float32)   # [4, 3, N1]
    p2T2 = (2.0 * np.transpose(p2, (0, 2, 1))).astype(np.float32)  # [4, 3, N2]
    p2sq = np.sum(p2.astype(np.float64) ** 2, -1)          # [4, N2]
    a1, b1_, c1_ = split3(p1T)
    x2, y2, z2 = split3(p2T2)
    s1_, s2_, s3_ = split3((-p2sq).astype(np.float32))
    p1sq = np.sum(p1.astype(np.float64) ** 2, -1) + EPS_DIST  # [4, N1]
    n1_, n2_, n3_ = split3((-p1sq).astype(np.float32))
    onesr = np.ones((BPC, 1, N1), ml_dtypes.bfloat16)
    onesc = np.ones((BPC, 1, N2), ml_dtypes.bfloat16)
    # device matmul output: q = 2*p1.p2 - |p2|^2 - (|p1|^2+eps) = -dist
    p1e = np.concatenate(
        [a1, a1, b1_, a1, b1_, c1_, onesr, onesr, onesr,
         n1_[:, None, :], n2_[:, None, :], n3_[:, None, :]], axis=1
    )  # [4, 24, N1]
    rhs4 = np.concatenate(
        [x2, y2, x2, z2, y2, x2,
         s1_[:, None, :], s2_[:, None, :], s3_[:, None, :],
         onesc, onesc, onesc], axis=1
    )  # [4, 24, N2]
    m = {
        "p1e": np.ascontiguousarray(p1e.astype(ml_dtypes.bfloat16)),
        "rhs4": np.ascontiguousarray(rhs4.astype(ml_dtypes.bfloat16)),
        "f1T": np.ascontiguousarray(
            np.transpose(f1, (0, 2, 1)).astype(ml_dtypes.bfloat16)
        ),
    }
    W1r = inputs["W1"][:, 0:C2]   # [H1, C2]
    W1fT = inputs["W1"][:, C2:].T  # [C1, H1]
    for b in range(BPC):
        g1b = f2[b].astype(np.float32) @ W1r.T.astype(np.float32)  # [N2, H1]
        m[f"g1_{b}"] = np.ascontiguousarray(g1b.astype(ml_dtypes.bfloat16))
    m["W1fT"] = np.ascontiguousarray(W1fT.astype(ml_dtypes.bfloat16))
    # shared weights
    s1 = inputs["g1"] / np.sqrt(inputs["v1"] + EPS_BN)
    b1f = (inputs["b1"] - inputs["m1"]) * s1 + inputs["be1"]
    s2 = inputs["g2"] / np.sqrt(inputs["v2"] + EPS_BN)
    b2f = (inputs["b2"] - inputs["m2"]) * s2 + inputs["be2"]
    m["W2T"] = np.ascontiguousarray(inputs["W2"].T.astype(ml_dtypes.bfloat16))
    m["sb1"] = np.ascontiguousarray(np.stack([s1, b1f], -1).astype(np.float32))
    m["sb2"] = np.ascontiguousarray(np.stack([s2, b2f], -1).astype(np.float32))
    return m


def run(inputs, trace=False):
    nc = _get_nc()
    in_maps = [_prep_core(inputs, c) for c in range(NCORES)]
    res = run_bass_kernel_spmd(
        nc, in_maps, core_ids=list(range(NCORES)), trace=trace
    )
    outs = [np.asarray(r["outT"]) for r in res.results]
    full = np.concatenate(outs, 0)          # [32, H2, N1]
    out = np.ascontiguousarray(np.transpose(full, (0, 2, 1)))  # [32, N1, H2]
    return out, res


def kernel(**inputs):
    out, _ = run(inputs, trace=False)
    return out


# revision 42
# speedup vs baseline: 1.0112x; 1.0112x over previous
"""Trainium2 Bass kernel for Memorynet (KNN-interp + 1x1-conv MLP).

Strategy: pure data parallel over batch (32 batches -> 8 cores x 4).
Per batch, per 128-token tile:
  S = 2*p1@p2.T - |p2|^2  (one K=21 bf16-split matmul into PSUM, [128 tok, 512 n2])
  top-8 S values via DVE max8 (top-3 used; m4 used for the mask threshold)
  Z = sum_k 1/(p1sq+eps-S_k); the normalized-weight row is built WITHOUT
  index extraction via the D' trick:
      D'_j  = Z*(p1sq+eps) - Z*S_j          (one ScalarE activation pass)
      tau   = Z*(p1sq+eps) - Z*(m3+m4)/2    (midpoint threshold, robust)
      A_j   = (D'_j <= tau) / D'_j          (one GpSimd fused pass -> bf16)
  so A_j = 1[S_j in top-3] * (1/dist_j)/Z exactly.
  A.T via DMA transpose (per 4-tile half); recvT-h1 accumulated in PSUM via
  G1c.T @ ATc matmuls (G1 = f2 @ W1r.T precomputed on host).
MLP is feature-major bf16; BN+ReLU folded into ScalarE activation.
Software-pipelined with depth-2 skew and fine-grain interleave.
"""

import sys

sys.path.insert(0, "/opt/trn_rl_repo")

import numpy as np
import ml_dtypes

import concourse.bass as bass
import concourse.bacc as bacc_mod
import concourse.mybir as mybir
from concourse.tile import TileContext
from concourse.bass_utils import run_bass_kernel_spmd

EPS_DIST = 1e-8  # matches reference; the Z-normalized reciprocal is
                 # self-protecting against fp32 rounding of tiny distances
EPS_BN = 1e-5
NCORES = 8
BPC = 4  # batches per core
N1, N2, C1, C2 = 2048, 512, 128, 256
CIN, H1, H2 = C1 + C2, 256, 128
NT = N1 // 128   # 16 token tiles / batch
PAIR = 8         # token tiles per KNN front (1024 tokens)
NPP = NT // PAIR # 2 pairs per batch

f32 = mybir.dt.float32
bf16 = mybir.dt.bfloat16
u32 = mybir.dt.uint32

AT = mybir.ActivationFunctionType
OP = mybir.AluOpType
AX = mybir.AxisListType


KS = 24  # split-matmul contraction rows: 18 products + 3*(-|p2|^2) + 3*(-(|p1|^2+eps))


def build_bass():
    nc = bacc_mod.Bacc()
    p1e = nc.declare_dram_parameter("p1e", [BPC, KS, N1], bf16, isOutput=False)
    rhs4 = nc.declare_dram_parameter("rhs4", [BPC, KS, N2], bf16, isOutput=False)
    f1T = nc.declare_dram_parameter("f1T", [BPC, C1, N1], bf16, isOutput=False)
    g1s = [
        nc.declare_dram_parameter(f"g1_{b}", [N2, H1], bf16, isOutput=False)
        for b in range(BPC)
    ]
    W1fd = nc.declare_dram_parameter("W1fT", [C1, H1], bf16, isOutput=False)
    W2Td = nc.declare_dram_parameter("W2T", [H1, H2], bf16, isOutput=False)
    sb1d = nc.declare_dram_parameter("sb1", [H1, 2], f32, isOutput=False)
    sb2d = nc.declare_dram_parameter("sb2", [H2, 2], f32, isOutput=False)
    outT = nc.declare_dram_parameter("outT", [BPC, H2, N1], f32, isOutput=True)

    with TileContext(nc) as tc:
        with (
            tc.tile_pool(name="const", bufs=1) as cpool,
            tc.tile_pool(name="batch", bufs=4) as bpool,
            tc.tile_pool(name="math", bufs=3) as gpool,
            tc.tile_pool(name="idxp", bufs=3) as idxpool,
            tc.tile_pool(name="dpp", bufs=4) as dppool,
            tc.tile_pool(name="at", bufs=3) as gkpool,
            tc.tile_pool(name="ab", bufs=3) as dpool,
            tc.tile_pool(name="xg", bufs=4) as xpool,
            tc.tile_pool(name="ps_s", bufs=5, space="PSUM") as ps_s,
            tc.tile_pool(name="ps_mlp", bufs=2, space="PSUM") as ps_mlp,
        ):
            # ---- constants ----
            W1f = cpool.tile([C1, H1], bf16)
            nc.sync.dma_start(out=W1f[:], in_=W1fd[:, :])
            W2T = [cpool.tile([128, H2], bf16, tag=f"w2_{k}", name=f"w2_{k}") for k in range(2)]
            for k in range(2):
                nc.sync.dma_start(out=W2T[k][:], in_=W2Td[128 * k:128 * (k + 1), :])
            sb1 = [cpool.tile([128, 2], f32, tag=f"sb1_{k}", name=f"sb1_{k}") for k in range(2)]
            for k in range(2):
                nc.sync.dma_start(out=sb1[k][:], in_=sb1d[128 * k:128 * (k + 1), :])
            sb2 = cpool.tile([128, 2], f32)
            nc.sync.dma_start(out=sb2[:], in_=sb2d[:, :])

            # ---- per-batch persistent inputs ----
            bstate = {}
            for b in range(BPC):
                p1eb = bpool.tile([KS, N1], bf16, tag="p1eb")
                nc.sync.dma_start(out=p1eb[:], in_=p1e[b, :, :])
                rhsb = bpool.tile([KS, N2], bf16, tag="rhsb")
                nc.sync.dma_start(out=rhsb[:], in_=rhs4[b, :, :])
                g1sb = bpool.tile([128, 4, H1], bf16, tag="g1sb")
                nc.sync.dma_start(
                    out=g1sb[:], in_=g1s[b][:, :].rearrange("(c p) d -> p c d", p=128)
                )
                f1b = bpool.tile([C1, N1], bf16, tag="f1b")
                nc.scalar.dma_start(out=f1b[:], in_=f1T[b, :, :])
                bstate[b] = (p1eb, rhsb, g1sb, f1b)

            def knn_tile(b, gp, t, maxg, sps):
                """One negated-distance matmul (out = -dist) + top-8."""
                p1eb, rhsb, _, _ = bstate[b]
                tau_ = PAIR * gp + t
                Sp = ps_s.tile([128, N2], f32, tag="Sp")
                nc.tensor.matmul(
                    out=Sp[:],
                    lhsT=p1eb[:, 128 * tau_:128 * (tau_ + 1)],
                    rhs=rhsb[:],
                    start=True,
                    stop=True,
                )
                nc.vector.max(out=maxg[:, t, :], in_=Sp[:])
                sps[t] = Sp

            def knn_half(b, gp, h, maxg, sps, ATt, Ag):
                """Weight math + reciprocal weights + mask for 4 tiles.

                maxg holds q = -dist (top-8, descending = 3 nearest first).
                Zneg = sum_k 1/q_k = -Z.  Scalar computes Rp = 1/(Zneg*q)
                = 1/(Z*d) for every position; DVE masks to the top-3 via
                q >= (q3+q4)/2.
                """
                sl = slice(4 * h, 4 * h + 4)
                recd = gpool.tile([128, 4, 3], f32, tag="recd")
                nc.vector.reciprocal(out=recd[:], in_=maxg[:, sl, 0:3])
                Zneg = gpool.tile([128, 4], f32, tag="Zneg")
                nc.vector.reduce_sum(out=Zneg[:], in_=recd[:], axis=AX.X)
                s34 = gpool.tile([128, 4], f32, tag="s34")
                nc.vector.tensor_tensor(
                    out=s34[:], in0=maxg[:, sl, 2], in1=maxg[:, sl, 3], op=OP.add
                )
                tau = gpool.tile([128, 4], f32, tag="tau")
                nc.vector.tensor_scalar_mul(tau[:], s34[:], 0.5)
                for j in range(4):
                    t = 4 * h + j
                    Rp = dppool.tile([128, N2], f32, tag="Rp")
                    nc.scalar.add_instruction(
                        mybir.InstActivation(
                            name=nc.scalar.bass.get_next_instruction_name(),
                            func=AT.Reciprocal,
                            ins=[
                                nc.scalar.lower_ap(sps[t][:]),
                                mybir.ImmediateValue(dtype=f32, value=0.0),
                                nc.scalar.lower_ap(Zneg[:, j:j + 1]),
                                mybir.ImmediateValue(dtype=f32, value=0.0),
                            ],
                            outs=[nc.scalar.lower_ap(Rp[:])],
                        )
                    )
                    # min(mask, Rp): selected -> Rp (weights are <= 1),
                    # masked -> 0; IEEE minNum suppresses NaN/inf from the
                    # reciprocal of near-zero/negative distances, yielding
                    # the correct w=1 limit for degenerate nearest points.
                    nc.vector.scalar_tensor_tensor(
                        out=Ag[:, t, :],
                        in0=sps[t][:],
                        scalar=tau[:, j:j + 1],
                        in1=Rp[:],
                        op0=OP.is_ge,
                        op1=OP.min,
                    )
                nc.sync.dma_start_transpose(
                    out=ATt[:, 16 * h:16 * (h + 1), :], in_=Ag[:, sl, :]
                )

            def mlp_chunks(b, gp, ATt):
                """6 issue-chunks: (g0,m0) (g0,m1) (g0,L2) (g1,m0) (g1,m1) (g1,L2)."""
                _, _, g1sb, f1b = bstate[b]
                ATv = ATt[:].rearrange("p (t c) r -> p c t r", c=4)
                chunks = []
                for gsub in range(2):
                    g = 2 * gp + gsub
                    h1 = [
                        xpool.tile([128, 512], bf16, tag=f"h1_{m}", name=f"h1_{m}")
                        for m in range(2)
                    ]

                    def l1(m, gsub=gsub, g=g, h1=h1):
                        l1p = ps_mlp.tile([128, 512], f32, tag="l1p")
                        nc.tensor.matmul(
                            out=l1p[:],
                            lhsT=W1f[:, 128 * m:128 * (m + 1)],
                            rhs=f1b[:, 512 * g:512 * (g + 1)],
                            start=True,
                            stop=False,
                        )
                        for c in range(4):
                            nc.tensor.matmul(
                                out=l1p[:],
                                lhsT=g1sb[:, c, 128 * m:128 * (m + 1)],
                                rhs=ATv[:, c, 4 * gsub:4 * gsub + 4, :],
                                start=False,
                                stop=(c == 3),
                            )
                        nc.scalar.activation(
                            out=h1[m][:],
                            in_=l1p[:],
                            func=AT.Relu,
                            scale=sb1[m][:, 0:1],
                            bias=sb1[m][:, 1:2],
                        )

                    def l2(g=g, h1=h1):
                        l2p = ps_mlp.tile([128, 512], f32, tag="l2p", bufs=1)
                        for kk in range(2):
                            nc.tensor.matmul(
                                out=l2p[:],
                                lhsT=W2T[kk][:],
                                rhs=h1[kk][:],
                                start=(kk == 0),
                                stop=(kk == 1),
                            )
                        o = xpool.tile([128, 512], f32, tag="osb")
                        nc.scalar.activation(
                            out=o[:],
                            in_=l2p[:],
                            func=AT.Relu,
                            scale=sb2[:, 0:1],
                            bias=sb2[:, 1:2],
                        )
                        nc.sync.dma_start(
                            out=outT[b, :, 512 * g:512 * (g + 1)], in_=o[:]
                        )

                    chunks.append(lambda l1=l1: l1(0))
                    chunks.append(lambda l1=l1: l1(1))
                    chunks.append(l2)
                return chunks

            # ---- software pipeline with fine-grain interleave ----
            iters = [(b, gp) for gp in range(NPP) for b in range(BPC)]
            DEPTH = 2
            pend = []
            ready = []
            for i, (b, gp) in enumerate(iters):
                maxg = idxpool.tile([128, PAIR, 8], f32, tag="maxg")
                sps = {}
                Ag = dpool.tile([128, PAIR, N2], bf16, tag="A")
                ATt = gkpool.tile([128, PAIR * 4, 128], bf16, tag="ATt")
                chunks = mlp_chunks(*pend[i - DEPTH]) if i >= DEPTH else []
                for t in range(4):
                    knn_tile(b, gp, t, maxg, sps)
                knn_half(b, gp, 0, maxg, sps, ATt, Ag)
                if chunks:
                    chunks[0]()
                    chunks[1]()
                knn_tile(b, gp, 4, maxg, sps)
                if chunks:
                    chunks[2]()
                knn_tile(b, gp, 5, maxg, sps)
                if chunks:
                    chunks[3]()
                knn_tile(b, gp, 6, maxg, sps)
                if chunks:
                    chunks[4]()
                knn_tile(b, gp, 7, maxg, sps)
                knn_half(b, gp, 1, maxg, sps, ATt, Ag)
                if chunks:
                    chunks[5]()
                pend.append((b, gp, ATt))
            flush = [mlp_chunks(*pend[j]) for j in range(len(iters) - DEPTH, len(iters))]
            for k in range(max(len(f) for f in flush)):
                for f in flush:
                    if k < len(f):
                        f[k]()

    nc.compile()
    return nc


_CACHE = {}


def _get_nc():
    if "nc" not in _CACHE:
        _CACHE["nc"] = build_bass()
    return _CACHE["nc"]


def _prep_core(inputs, c):
    """Host-side prep of one core's input map (batches 4c..4c+4)."""
    sl = slice(BPC * c, BPC * (c + 1))
    p1 = inputs["points_1"][sl]     # [4, N1, 3]
    p2 = inputs["points_2"][sl]     # [4, N2, 3]
    f1 = inputs["features_1"][sl]   # [4, N1, C1]
    f2 = inputs["features_2"][sl]   # [4, N2, C2]

    def split3(x):
        a = x.astype(ml_dtypes.bfloat16)
        r = x - a.astype(np.float32)
        bb = r.astype(ml_dtypes.bfloat16)
        cc = (r - bb.astype(np.float32)).astype(ml_dtypes.bfloat16)
        return a, bb, cc

    p1T = np.transpose(p1, (0, 2, 1)).astype(np.float32)   # [4, 3, N1]
    p2T2 = (2.0 * np.transpose(p2, (0, 2, 1))).astype(np.float32)  # [4, 3, N2]
    p2sq = np.sum(p2.astype(np.float64) ** 2, -1)          # [4, N2]
    a1, b1_, c1_ = split3(p1T)
    x2, y2, z2 = split3(p2T2)
    s1_, s2_, s3_ = split3((-p2sq).astype(np.float32))
    p1sq = np.sum(p1.astype(np.float64) ** 2, -1) + EPS_DIST  # [4, N1]
    n1_, n2_, n3_ = split3((-p1sq).astype(np.float32))
    onesr = np.ones((BPC, 1, N1), ml_dtypes.bfloat16)
    onesc = np.ones((BPC, 1, N2), ml_dtypes.bfloat16)
    # device matmul output: q = 2*p1.p2 - |p2|^2 - (|p1|^2+eps) = -dist
    p1e = np.concatenate(
        [a1, a1, b1_, a1, b1_, c1_, onesr, onesr, onesr,
         n1_[:, None, :], n2_[:, None, :], n3_[:, None, :]], axis=1
    )  # [4, 24, N1]
    rhs4 = np.concatenate(
        [x2, y2, x2, z2, y2, x2,
         s1_[:, None, :], s2_[:, None, :], s3_[:, None, :],
         onesc, onesc, onesc], axis=1
    )  # [4, 24, N2]
    m = {
        "p1e": np.ascontiguousarray(p1e.astype(ml_dtypes.bfloat16)),
        "rhs4": np.ascontiguousarray(rhs4.astype(ml_dtypes.bfloat16)),
        "f1T": np.ascontiguousarray(
            np.transpose(f1, (0, 2, 1)).astype(ml_dtypes.bfloat16)
        ),
    }
    W1r = inputs["W1"][:, 0:C2]   # [H1, C2]
    W1fT = inputs["W1"][:, C2:].T  # [C1, H1]
    for b in range(BPC):
        g1b = f2[b].astype(np.float32) @ W1r.T.astype(np.float32)  # [N2, H1]
        m[f"g1_{b}"] = np.ascontiguousarray(g1b.astype(ml_dtypes.bfloat16))
    m["W1fT"] = np.ascontiguousarray(W1fT.astype(ml_dtypes.bfloat16))
    # shared weights
    s1 = inputs["g1"] / np.sqrt(inputs["v1"] + EPS_BN)
    b1f = (inputs["b1"] - inputs["m1"]) * s1 + inputs["be1"]
    s2 = inputs["g2"] / np.sqrt(inputs["v2"] + EPS_BN)
    b2f = (inputs["b2"] - inputs["m2"]) * s2 + inputs["be2"]
    m["W2T"] = np.ascontiguousarray(inputs["W2"].T.astype(ml_dtypes.bfloat16))
    m["sb1"] = np.ascontiguousarray(np.stack([s1, b1f], -1).astype(np.float32))
    m["sb2"] = np.ascontiguousarray(np.stack([s2, b2f], -1).astype(np.float32))
    return m


def run(inputs, trace=False):
    nc = _get_nc()
    in_maps = [_prep_core(inputs, c) for c in range(NCORES)]
    res = run_bass_kernel_spmd(
        nc, in_maps, core_ids=list(range(NCORES)), trace=trace
    )
    outs = [np.asarray(r["outT"]) for r in res.results]
    full = np.concatenate(outs, 0)          # [32, H2, N1]
    out = np.ascontiguousarray(np.transpose(full, (0, 2, 1)))  # [32, N1, H2]
    return out, res


def kernel(**inputs):
    out, _ = run(inputs, trace=False)
    return out
